# revision 19
# baseline (speedup 1.0000x reference)
"""Trainium2 Bass kernel for a dense transformer block (B=8, N=1024, C=1024,
H=16, D=64, HID=4096) with padding-masked attention.

Sharding: data-parallel over batch — one batch element per NeuronCore (8 cores).
Each core runs the full block on its [N, C] slice in a transposed layout
([C partitions, N free]) so every matmul contracts over the partition dim;
LayerNorm stats and softmax denominators come from ones-augmented matmuls.

All matmul operands are float32r (TF32-style full-rate PE mode, 1 cyc/row for
moving dims >= 256; measured end-to-end rel err ~1.4e-4).

DMA discipline: transfers are merged into [128, >=2K] descriptors (~1.3us of
queue time per dma_start otherwise dominates) and spread across the SP/ACT/
DVE/Pool DGE queues.
"""

import os
import sys

for _p in ("/opt/trn_rl_repo",):
    if _p not in sys.path:
        sys.path.insert(0, _p)
os.environ.setdefault("MYCRO_LOCAL_CACHE", "1")

import numpy as np  # noqa: E402

import concourse.bacc as bacc  # noqa: E402
import concourse.tile as tile  # noqa: E402
from concourse import mybir  # noqa: E402
from concourse.bass_utils import run_bass_kernel_spmd  # noqa: E402

f32 = mybir.dt.float32
f32r = mybir.dt.float32r
AF = mybir.ActivationFunctionType
ALU = mybir.AluOpType

B, N, C = 8, 1024, 1024
H, D = 16, 64
HID = 4 * C
CT = C // 128          # 8 c-tiles
NT = N // 128          # 8 n/k-tiles
HT = HID // 128        # 32 hid-tiles
SCALE = D ** -0.5
EPS = 1e-5
MASK_NEG = -10000.0

NCORES = 8


def _layer_norm(nc, tc, srcf, dst, onesP, epsc, gc, bc, tag):
    """dst[ct] = LN(src) per token (free dim); `srcf(ct)` returns the ct'th
    [128, N] slice (C on partitions).  Stats via ones-matmul partition
    reductions, broadcast to all 128 partitions."""
    ln_cm = tc.tile_pool(name=f"ln_{tag}", bufs=1)
    lnps_cm = tc.tile_pool(name=f"lnps_{tag}", bufs=1, space="PSUM")
    work = ln_cm.__enter__()
    ps = lnps_cm.__enter__()

    ps_sum = ps.tile([128, N], f32, tag="lnsum", name=f"ps_sum_{tag}")
    ps_sq = ps.tile([128, N], f32, tag="lnsq", name=f"ps_sq_{tag}")
    sq_tiles = []
    for ct in range(CT):
        sq = work.tile([128, N], f32r, tag="lnsqt", bufs=CT, name=f"sq{tag}{ct}")
        nc.scalar.activation(out=sq, in_=srcf(ct), func=AF.Square)
        sq_tiles.append(sq)
    for ch in range(2):
        cs = slice(ch * 512, (ch + 1) * 512)
        for ct in range(CT):
            nc.tensor.matmul(ps_sum[:, cs], lhsT=onesP, rhs=srcf(ct)[:, cs],
                             start=(ct == 0), stop=(ct == CT - 1))
        for ct in range(CT):
            nc.tensor.matmul(ps_sq[:, cs], lhsT=onesP, rhs=sq_tiles[ct][:, cs],
                             start=(ct == 0), stop=(ct == CT - 1))
    meanB = work.tile([128, N], f32, tag="meanB", name=f"meanB_{tag}")
    nc.vector.tensor_scalar_mul(meanB, ps_sum, 1.0 / C)
    msq = work.tile([128, N], f32, tag="msq", name=f"msq_{tag}")
    nc.vector.tensor_mul(msq, meanB, meanB)
    varB = work.tile([128, N], f32, tag="varB", name=f"varB_{tag}")
    nc.vector.scalar_tensor_tensor(varB, in0=ps_sq, scalar=1.0 / C, in1=msq,
                                   op0=ALU.mult, op1=ALU.subtract)
    stdB = work.tile([128, N], f32, tag="stdB", name=f"stdB_{tag}")
    nc.scalar.activation(out=stdB, in_=varB, func=AF.Sqrt, bias=epsc, scale=1.0)
    rstdB = work.tile([128, N], f32, tag="rstdB", name=f"rstdB_{tag}")
    nc.vector.reciprocal(rstdB, stdB)
    for ct in range(CT):
        d = work.tile([128, N], f32, tag="lnd", bufs=3, name=f"lnd{tag}{ct}")
        nc.vector.tensor_sub(d, srcf(ct), meanB)
        t = work.tile([128, N], f32, tag="lnt", bufs=3, name=f"lnt{tag}{ct}")
        nc.vector.scalar_tensor_tensor(t, in0=d, scalar=gc[:, ct:ct + 1],
                                       in1=rstdB, op0=ALU.mult, op1=ALU.mult)
        nc.scalar.activation(out=dst[ct], in_=t, func=AF.Identity,
                             bias=bc[:, ct:ct + 1], scale=1.0)

    lnps_cm.__exit__(None, None, None)
    ln_cm.__exit__(None, None, None)


def build_program(repeat=1):
    nc = bacc.Bacc("TRN2", target_bir_lowering=False, debug=False)

    xT = nc.dram_tensor("xT", [C, N], f32, kind="ExternalInput").ap()
    maskv = nc.dram_tensor("maskv", [N], f32, kind="ExternalInput").ap()
    g1 = nc.dram_tensor("g1", [C], f32, kind="ExternalInput").ap()
    b1 = nc.dram_tensor("b1", [C], f32, kind="ExternalInput").ap()
    g2 = nc.dram_tensor("g2", [C], f32, kind="ExternalInput").ap()
    b2 = nc.dram_tensor("b2", [C], f32, kind="ExternalInput").ap()
    bproj = nc.dram_tensor("bproj", [C], f32, kind="ExternalInput").ap()
    bb1 = nc.dram_tensor("bb1", [HID], f32, kind="ExternalInput").ap()
    bb2 = nc.dram_tensor("bb2", [C], f32, kind="ExternalInput").ap()
    wqkv = nc.dram_tensor("wqkv", [6, CT, 128, 512], f32, kind="ExternalInput").ap()
    wproj = nc.dram_tensor("wproj", [2, CT, 128, 512], f32, kind="ExternalInput").ap()
    w1 = nc.dram_tensor("w1", [8, CT, 128, 512], f32, kind="ExternalInput").ap()
    w2 = nc.dram_tensor("w2", [2, HT, 128, 512], f32, kind="ExternalInput").ap()
    onesd = nc.dram_tensor("onesd", [1, 128], f32, kind="ExternalInput").ap()
    outT = nc.dram_tensor("outT", [C, N], f32, kind="ExternalOutput").ap()
    x2d = nc.dram_tensor("x2d", [C, N], f32)  # internal spill of attn output
    rrd = nc.dram_tensor("rrd", [H, N], f32)  # denom-reciprocal bounce

    # [C, N] DRAM tensors viewed as two [128, 4, N] row-groups for merged DMA
    def rg(ap_, half):
        return ap_[half * 512:(half + 1) * 512, :].rearrange(
            "(a p) f -> p a f", p=128)

    with tile.TileContext(nc) as tc:
        const_cm = tc.tile_pool(name="const", bufs=1)
        const = const_cm.__enter__()

        def vec_tiles(src_ap, n_t, name):
            t = const.tile([128, n_t], f32, name=name)
            nc.sync.dma_start(out=t, in_=src_ap.rearrange("(t p) -> p t", p=128))
            return t

        g1c = vec_tiles(g1, CT, "g1c")
        b1c = vec_tiles(b1, CT, "b1c")
        g2c = vec_tiles(g2, CT, "g2c")
        b2c = vec_tiles(b2, CT, "b2c")
        bprojc = vec_tiles(bproj, CT, "bprojc")
        bb1c = vec_tiles(bb1, HT, "bb1c")
        bb2c = vec_tiles(bb2, CT, "bb2c")
        maskc = vec_tiles(maskv, NT, "maskc")
        onesP = const.tile([128, 128], f32r, name="onesP")
        nc.sync.dma_start(out=onesP,
                          in_=onesd.partition_broadcast(128).bitcast(f32r))
        epsc = const.tile([128, 1], f32, name="epsc")
        nc.vector.memset(epsc, EPS)

        for _rep in range(repeat):
            # ==================== LN1 (x -> xn) ======================
            pln1_cm = tc.tile_pool(name="p_ln1", bufs=1, side="left")
            pln1 = pln1_cm.__enter__()
            xn = [pln1.tile([128, N], f32r, tag="xn", bufs=CT, name=f"xn{ct}")
                  for ct in range(CT)]
            pxts_cm = tc.tile_pool(name="p_xts", bufs=1, side="left")
            pxts = pxts_cm.__enter__()
            xtsB = []
            for g in range(2):
                t = pxts.tile([128, 4, N], f32r, tag="xts", bufs=2,
                              name=f"xts{g}")
                nc.scalar.dma_start(out=t, in_=rg(xT, g).bitcast(f32r))
                xtsB.append(t)

            def xslice(ct):
                return xtsB[ct // 4][:, ct % 4, :]

            _layer_norm(nc, tc, xslice, xn, onesP, epsc, g1c, b1c, "ln1")
            pxts_cm.__exit__(None, None, None)

            # ======================== QKV ============================
            pattn_cm = tc.tile_pool(name="p_attn", bufs=1, side="right")
            pattn = pattn_cm.__enter__()
            qkt = [pattn.tile([128, N], f32r, tag="qkt", bufs=16,
                              name=f"qkt{i}") for i in range(16)]
            # vkt: per k-tile [v_h0 | 1 | v_h1 | 1 | ...] -> [128, 16*65]
            vkt = [pattn.tile([128, H * (D + 1)], f32r, tag="vkt", bufs=NT,
                              name=f"vkt{kt}") for kt in range(NT)]
            ones16 = onesd[0:1, 0:16].partition_broadcast(128).rearrange(
                "p a (h o) -> p (a h) o", o=1).bitcast(f32r)
            for kt in range(NT):
                vcol = vkt[kt].rearrange("p (h u) -> p h u", u=D + 1)
                nc.gpsimd.dma_start(out=vcol[:, :, D:D + 1], in_=ones16)

            wq_cm = tc.tile_pool(name="wq_pool", bufs=1)
            wqp = wq_cm.__enter__()
            qps_cm = tc.tile_pool(name="qkv_ps", bufs=1, space="PSUM")
            qps = qps_cm.__enter__()

            # weights stream: [128, 4, 512] per (ftg, ct-group)
            wtiles = {}
            for ftg in range(6):
                for cg in range(2):
                    wt = wqp.tile([128, 4, 512], f32r, tag="wqkv", bufs=6,
                                  name=f"wq{ftg}_{cg}")
                    nc.sync.dma_start(
                        out=wt,
                        in_=wqkv[ftg, 4 * cg:4 * cg + 4].rearrange(
                            "a p f -> p a f").bitcast(f32r))
                    wtiles[(ftg, cg)] = wt

            def wslice(ftg, ct, fs):
                return wtiles[(ftg, ct // 4)][:, ct % 4, fs]

            for ftg in range(4):      # q then k feature tiles
                for ft in range(4):
                    ps = qps.tile([128, N], f32, tag="qkvps", bufs=3,
                                  name=f"qkps{ftg}_{ft}")
                    fs = slice(ft * 128, (ft + 1) * 128)
                    for ct in range(CT):
                        for ch in range(2):
                            cs = slice(ch * 512, (ch + 1) * 512)
                            nc.tensor.matmul(
                                ps[:, cs], lhsT=wslice(ftg, ct, fs),
                                rhs=xn[ct][:, cs],
                                start=(ct == 0), stop=(ct == CT - 1))
                    nc.scalar.copy(out=qkt[ftg * 4 + ft], in_=ps)
            for nt in range(NT):      # v in natural layout
                ps = qps.tile([128, N], f32, tag="qkvps", bufs=3,
                              name=f"vps{nt}")
                for ct in range(CT):
                    for ch in range(2):
                        cs = slice(ch * 512, (ch + 1) * 512)
                        nc.tensor.matmul(
                            ps[:, cs],
                            lhsT=xn[ct][:, nt * 128:(nt + 1) * 128],
                            rhs=wtiles[(4 + ch, ct // 4)][:, ct % 4, :],
                            start=(ct == 0), stop=(ct == CT - 1))
                for h in range(H):
                    nc.vector.tensor_copy(
                        vkt[nt][:, h * 65:h * 65 + D],
                        ps[:, h * D:(h + 1) * D])

            qps_cm.__exit__(None, None, None)
            wq_cm.__exit__(None, None, None)
            pln1_cm.__exit__(None, None, None)

            # ====================== attention ========================
            pyt_cm = tc.tile_pool(name="p_yt", bufs=1, side="left")
            pyt = pyt_cm.__enter__()
            yt = [pyt.tile([128, N], f32r, tag="yt", bufs=NT, name=f"yt{j}")
                  for j in range(NT)]
            asb_cm = tc.tile_pool(name="attn_sb", bufs=1)
            asb = asb_cm.__enter__()
            aps_cm = tc.tile_pool(name="attn_ps", bufs=1, space="PSUM")
            aps = aps_cm.__enter__()

            for j in range(8):  # head pairs (2j, 2j+1)
                qk_q = qkt[j]
                qk_k = qkt[8 + j]
                ya = aps.tile([D + 1, N], f32, tag="ya", name=f"ya{j}")
                yb = aps.tile([D + 1, N], f32, tag="yb", name=f"yb{j}")
                for kt in range(NT):
                    sa = aps.tile([128, N], f32, tag="sa", name=f"sa{j}_{kt}")
                    sb_ = aps.tile([128, N], f32, tag="sb", name=f"sb{j}_{kt}")
                    ks = slice(kt * 128, (kt + 1) * 128)
                    for ch in range(2):
                        cs = slice(ch * 512, (ch + 1) * 512)
                        nc.tensor.matmul(sa[:, cs], lhsT=qk_k[0:D, ks],
                                         rhs=qk_q[0:D, cs],
                                         start=True, stop=True)
                        nc.tensor.matmul(sb_[:, cs], lhsT=qk_k[D:128, ks],
                                         rhs=qk_q[D:128, cs],
                                         start=True, stop=True)
                    ea = asb.tile([128, N], f32r, tag="ea", bufs=2,
                                  name=f"ea{j}_{kt}")
                    eb = asb.tile([128, N], f32r, tag="eb", bufs=2,
                                  name=f"eb{j}_{kt}")
                    nc.scalar.activation(out=ea, in_=sa, func=AF.Exp,
                                         bias=maskc[:, kt:kt + 1], scale=SCALE)
                    nc.scalar.activation(out=eb, in_=sb_, func=AF.Exp,
                                         bias=maskc[:, kt:kt + 1], scale=SCALE)
                    for ch in range(2):
                        cs = slice(ch * 512, (ch + 1) * 512)
                        va = vkt[kt][:, 2 * j * 65:2 * j * 65 + 65]
                        vb = vkt[kt][:, (2 * j + 1) * 65:(2 * j + 1) * 65 + 65]
                        nc.tensor.matmul(ya[:, cs], lhsT=va, rhs=ea[:, cs],
                                         start=(kt == 0), stop=(kt == NT - 1))
                        nc.tensor.matmul(yb[:, cs], lhsT=vb, rhs=eb[:, cs],
                                         start=(kt == 0), stop=(kt == NT - 1))
                for half, yp in ((0, ya), (1, yb)):
                    yu = asb.tile([D + 1, N], f32, tag="yu", bufs=4,
                                  name=f"yu{j}_{half}")
                    nc.vector.tensor_copy(yu, yp)
                    rr = asb.tile([1, N], f32, tag="rr", bufs=2,
                                  name=f"rr{j}_{half}")
                    nc.vector.reciprocal(rr, yu[D:D + 1, :])
                    row = rrd.ap()[2 * j + half:2 * j + half + 1, :]
                    nc.scalar.dma_start(out=row, in_=rr)
                    rb = asb.tile([D, N], f32, tag="rb", bufs=2,
                                  name=f"rb{j}_{half}")
                    nc.scalar.dma_start(out=rb, in_=row.partition_broadcast(D))
                    nc.vector.tensor_mul(yt[j][half * D:(half + 1) * D, :],
                                         yu[0:D, :], rb)

            aps_cm.__exit__(None, None, None)
            asb_cm.__exit__(None, None, None)
            pattn_cm.__exit__(None, None, None)

            # =================== proj + residual =====================
            px2t_cm = tc.tile_pool(name="p_x2t", bufs=1, side="right")
            px2t = px2t_cm.__enter__()
            x2tB = [px2t.tile([128, 4, N], f32r, tag="x2t", bufs=2,
                              name=f"x2t{g}") for g in range(2)]

            def x2slice(ct):
                return x2tB[ct // 4][:, ct % 4, :]

            wp_cm = tc.tile_pool(name="wp_pool", bufs=1)
            wpp = wp_cm.__enter__()
            pps_cm = tc.tile_pool(name="proj_ps", bufs=1, space="PSUM")
            pps = pps_cm.__enter__()

            wptiles = {}
            for fg in range(2):
                for cg in range(2):
                    wt = wpp.tile([128, 4, 512], f32r, tag="wproj", bufs=4,
                                  name=f"wp{fg}_{cg}")
                    nc.sync.dma_start(
                        out=wt,
                        in_=wproj[fg, 4 * cg:4 * cg + 4].rearrange(
                            "a p f -> p a f").bitcast(f32r))
                    wptiles[(fg, cg)] = wt
            xr1 = []
            for g in range(2):
                xr = wpp.tile([128, 4, N], f32, tag="xr1", bufs=2,
                              name=f"xr1_{g}")
                nc.scalar.dma_start(out=xr, in_=rg(xT, g))
                xr1.append(xr)
            for o in range(CT):
                ps = pps.tile([128, N], f32, tag="projps", bufs=3,
                              name=f"pps{o}")
                fs = slice((o % 4) * 128, (o % 4 + 1) * 128)
                for ct in range(CT):
                    for ch in range(2):
                        cs = slice(ch * 512, (ch + 1) * 512)
                        nc.tensor.matmul(
                            ps[:, cs],
                            lhsT=wptiles[(o // 4, ct // 4)][:, ct % 4, fs],
                            rhs=yt[ct][:, cs],
                            start=(ct == 0), stop=(ct == CT - 1))
                # x2 = ps + bproj + x
                nc.vector.scalar_tensor_tensor(
                    x2slice(o), in0=ps, scalar=bprojc[:, o:o + 1],
                    in1=xr1[o // 4][:, o % 4, :], op0=ALU.add, op1=ALU.add)
            for g in range(2):
                nc.scalar.dma_start(out=rg(x2d.ap(), g).bitcast(f32r),
                                    in_=x2tB[g])

            pps_cm.__exit__(None, None, None)
            wp_cm.__exit__(None, None, None)
            pyt_cm.__exit__(None, None, None)

            # =================== LN2 (x2 -> x2n) =====================
            px2n_cm = tc.tile_pool(name="p_x2n", bufs=1, side="left")
            px2n = px2n_cm.__enter__()
            x2n = [px2n.tile([128, N], f32r, tag="x2n", bufs=CT,
                             name=f"x2n{ct}") for ct in range(CT)]
            _layer_norm(nc, tc, x2slice, x2n, onesP, epsc, g2c, b2c, "ln2")
            px2t_cm.__exit__(None, None, None)

            # ================== MLP (grouped 8-f) ====================
            pmlp_cm = tc.tile_pool(name="p_mlp", bufs=1, side="right")
            pmlp = pmlp_cm.__enter__()
            macc = [pmlp.tile([128, N], f32, tag="macc", bufs=CT,
                              name=f"macc{o}") for o in range(CT)]
            ht_all = [pmlp.tile([128, N], f32r, tag="ht", bufs=12,
                                name=f"ht{f}") for f in range(HT)]
            mw_cm = tc.tile_pool(name="mw_pool", bufs=1)
            mwp = mw_cm.__enter__()
            mps_cm = tc.tile_pool(name="mlp_ps", bufs=1, space="PSUM")
            mps = mps_cm.__enter__()

            NG = 4
            FP = HT // NG  # 8 f-tiles per group
            for g in range(NG):
                htg = []
                w1tiles = None
                for fl in range(FP):
                    f = g * FP + fl
                    fg, fi = f // 4, f % 4
                    if fi == 0:
                        w1tiles = []
                        for cg in range(2):
                            wt = mwp.tile([128, 4, 512], f32r, tag="w1",
                                          bufs=3, name=f"w1_{fg}_{cg}")
                            nc.sync.dma_start(
                                out=wt,
                                in_=w1[fg, 4 * cg:4 * cg + 4].rearrange(
                                    "a p f -> p a f").bitcast(f32r))
                            w1tiles.append(wt)
                    ps = mps.tile([128, N], f32, tag="mlp1ps", bufs=2,
                                  name=f"m1ps{f}")
                    fs = slice(fi * 128, (fi + 1) * 128)
                    for ct in range(CT):
                        for ch in range(2):
                            cs = slice(ch * 512, (ch + 1) * 512)
                            nc.tensor.matmul(
                                ps[:, cs],
                                lhsT=w1tiles[ct // 4][:, ct % 4, fs],
                                rhs=x2n[ct][:, cs],
                                start=(ct == 0), stop=(ct == CT - 1))
                    h = ht_all[f]
                    nc.scalar.activation(out=h, in_=ps, func=AF.Gelu,
                                         bias=bb1c[:, f:f + 1], scale=1.0)
                    htg.append(h)
                # W2 partial for this group of 8 f-rows, oh-major
                for oh in range(2):
                    w2tiles = []
                    for fgp in range(2):
                        wt = mwp.tile([128, 4, 512], f32r, tag="w2", bufs=3,
                                      name=f"w2_{g}_{oh}_{fgp}")
                        nc.sync.dma_start(
                            out=wt,
                            in_=w2[oh, g * FP + 4 * fgp:
                                   g * FP + 4 * fgp + 4].rearrange(
                                "a p f -> p a f").bitcast(f32r))
                        w2tiles.append(wt)
                    for o in range(4 * oh, 4 * oh + 4):
                        pm = mps.tile([128, N], f32, tag="pm", bufs=2,
                                      name=f"pm{g}_{o}")
                        fs = slice((o % 4) * 128, (o % 4 + 1) * 128)
                        for fl in range(FP):
                            for ch in range(2):
                                cs = slice(ch * 512, (ch + 1) * 512)
                                nc.tensor.matmul(
                                    pm[:, cs],
                                    lhsT=w2tiles[fl // 4][:, fl % 4, fs],
                                    rhs=htg[fl][:, cs],
                                    start=(fl == 0), stop=(fl == FP - 1))
                        if g == 0:
                            nc.vector.tensor_copy(macc[o], pm)
                        else:
                            nc.vector.tensor_add(macc[o], macc[o], pm)

            mps_cm.__exit__(None, None, None)
            mw_cm.__exit__(None, None, None)
            px2n_cm.__exit__(None, None, None)

            # ================ MLP2 bias + residual out ===============
            osb_cm = tc.tile_pool(name="out_sb", bufs=1)
            osb = osb_cm.__enter__()
            for g in range(2):
                xr = osb.tile([128, 4, N], f32, tag="xr", bufs=2,
                              name=f"xr{g}")
                nc.scalar.dma_start(out=xr, in_=rg(x2d.ap(), g))
                ot = osb.tile([128, 4, N], f32, tag="ot", bufs=2,
                              name=f"ot{g}")
                for o4 in range(4):
                    o = g * 4 + o4
                    nc.vector.scalar_tensor_tensor(
                        ot[:, o4, :], in0=macc[o], scalar=bb2c[:, o:o + 1],
                        in1=xr[:, o4, :], op0=ALU.add, op1=ALU.add)
                nc.scalar.dma_start(out=rg(outT, g), in_=ot)
            osb_cm.__exit__(None, None, None)
            pmlp_cm.__exit__(None, None, None)

        const_cm.__exit__(None, None, None)

    nc.compile()
    return nc


_NC_CACHE = {}


def _get_program():
    if "nc" not in _NC_CACHE:
        _NC_CACHE["nc"] = build_program()
    return _NC_CACHE["nc"]


def _prep_weights(Wqkv, Wproj, W1, W2):
    def til(WT, n_fg):
        # WT: [K, M] (contraction-major); -> [n_fg, K//128, 128, 512]
        K, M = WT.shape
        return np.ascontiguousarray(
            WT.reshape(K // 128, 128, n_fg, 512).transpose(2, 0, 1, 3))

    return {
        "wqkv": til(np.ascontiguousarray(Wqkv.T), 6),
        "wproj": til(np.ascontiguousarray(Wproj.T), 2),
        "w1": til(np.ascontiguousarray(W1.T), 8),
        "w2": til(np.ascontiguousarray(W2.T), 2),
    }


def kernel(x, length, g1, b1, Wqkv, Wproj, bproj, g2, b2, W1, bb1, W2, bb2):
    x = np.asarray(x, dtype=np.float32)
    length = np.asarray(length)
    g1 = np.asarray(g1, np.float32); b1 = np.asarray(b1, np.float32)
    g2 = np.asarray(g2, np.float32); b2 = np.asarray(b2, np.float32)
    bproj = np.asarray(bproj, np.float32)
    bb1 = np.asarray(bb1, np.float32); bb2 = np.asarray(bb2, np.float32)
    Wqkv = np.asarray(Wqkv, np.float32); Wproj = np.asarray(Wproj, np.float32)
    W1 = np.asarray(W1, np.float32); W2 = np.asarray(W2, np.float32)

    wts = _prep_weights(Wqkv, Wproj, W1, W2)
    xT = np.ascontiguousarray(x.transpose(0, 2, 1))  # [B, C, N]
    mask = (np.arange(N)[None, :] >= np.asarray(length)[:, None]).astype(
        np.float32) * MASK_NEG  # [B, N]

    shared = {"g1": g1, "b1": b1, "g2": g2, "b2": b2, "bproj": bproj,
              "bb1": bb1, "bb2": bb2, "onesd": np.ones((1, 128), np.float32),
              **wts}
    in_maps = [dict(shared, xT=xT[b], maskv=np.ascontiguousarray(mask[b]))
               for b in range(B)]

    nc = _get_program()
    res = run_bass_kernel_spmd(nc, in_maps, core_ids=list(range(NCORES)))
    out = np.stack([res.results[b]["outT"] for b in range(B)], axis=0)
    return np.ascontiguousarray(out.transpose(0, 2, 1))
